# revision 1
# baseline (speedup 1.0000x reference)
"""Trainium2 Bass kernel for nn_Attn (Bahdanau-style attention scores).

Computation (per batch b of B=128):
    energy = tanh(enc[b] @ We.T + (hidden @ Wh.T)[b] + bias)   # (L, H)
    scores = energy @ v                                        # (L,)
    out[b] = softmax(scores)                                   # (1, L)

Sharding: batch data-parallel over 8 NeuronCores (16 batches/core);
weights replicated. Per core the dominant matmul is computed in the
[h, l] orientation so the PE tensor engine contracts over d (=576):

    part_e[h, l] = sum_d WeT[d, h] * encT[d, l]      (lhsT=WeT, rhs=encT)

which lets the (hidden@Wh.T + bias) term fuse into the tanh as a
per-partition activation bias, and the v-contraction run as a second
PE matmul (lhsT = v column, contracting over h on partitions).
Matmuls run as float32r (full fp32 data, reduced-precision multiply,
1 col/cycle on TRN2 vs 4 for exact fp32).

The contraction dim is zero-padded host-side from 576 to 640 so every
k-tile is a full 128 partitions: K=64 matmuls (and their successors)
measure ~2x slower on HW than K=128 ones, costing far more than the 11%
extra DMA.

Host side: encoder_outputs (L, B, D) is transposed once to (B, D, L) so
each per-batch d-major tile DMA is contiguous.

Scores are assembled batch-major ([16, L] via tiny SBUF->SBUF row DMAs)
so softmax runs once over all local batches at the end instead of as 16
serial per-batch chains on the ACT/DVE engines.

Prologue schedule: the PE warms up on a DVE-memset tile (no DMA
dependency — starts at the Tensor engine's ~8us fixed wake-up instead
of ~10.5us waiting out the wet-DMA semaphore); DMAs are issued
wet -> enc[b0] -> small consts -> wht/hid, and batch 0's first l-half
matmul group is emitted BEFORE the c-block, so stage-1 begins as soon
as enc[b0] lands (~17us) rather than after every constant (~26us).
Batch 0's first group issues kt-major so each k-tile's 4 matmuls run
as soon as that tile's DMA lands. The c-block operands (wht/hid) are
float32r: fp32-exact matmuls lower to 2 HW instructions each (visible
as 32 slices for 16 emissions in the trace), f32r to 1. The prologue
is now DMA-throughput-bound (cumulative wet+enc[b0]+consts arrival),
not ordering-bound.

Measured dead ends (each slower on HW than this float32r version):
dt2=bf16 energy tiles (no stage-1 recovery — the 242-vs-227ns in-kernel
overage is not SBUF-read contention — and slower bf16 ones-matmuls,
with 15x less correctness margin);
bf16 stage-1 operands (268ns/matmul vs 227ns — slower weight-load
path, and DMA was never the binding constraint); fp8e4 DoubleRow
(2x per k-tile, but a hi/lo split to pass the 2e-2 gate needs 2-3x
the products; single fp8 sims at 2.4e-2); per-batch softmax straight
from the scores PSUM (stalls the in-order ACT queue behind the DVE
z-chain). Note: HW exec time for identical code drifts 204-246us with
device clock state; compare variants by trace structure, not wall.
"""

import numpy as np

import concourse.bacc as bacc
import concourse.bass as bass
import concourse.mybir as mybir
import concourse.tile as tile
from concourse import bass_utils
from concourse.mybir import ActivationFunctionType as AF
from concourse.mybir import AluOpType, AxisListType

N_CORES = 8
B, L, H = 128, 1024, 512
ONEHOT = 64
DE = H + ONEHOT          # 576, true contraction dim of the big matmul
DP = 640                 # padded contraction dim (5 full 128-tiles)
BL = B // N_CORES        # 16 batches per core
F32 = mybir.dt.float32
F32R = mybir.dt.float32r

NKT = DP // 128                          # 5 d-tiles, all full
NHT = H // 128                           # 4 h-tiles
NLH = L // 512                           # 2 l-halves (N=512 per matmul)


BF16 = mybir.dt.bfloat16


def build(reps: int = 1, dt1=F32R, dt2=F32R, dve2: bool = True):
    """Build + trace the per-core Bass program. Returns the compiled nc.

    dt1: dtype of the stage-1 matmul operands (enc tiles + We tiles).
    dt2: dtype of the stage-2 operands (energy tiles + v columns).
    dve2: compute z[p,l] = sum_ht v_ht[p]*en_ht[p,l] on the VectorE
        (per-partition scalar multiply-accumulate), so stage-2 on the PE
        collapses from 4 matmuls to a single K=128 ones-matmul per
        (batch, l-half). Saves ~23us of PE time for ~45us of idle DVE.
    """
    nc = bacc.Bacc(
        "TRN2", target_bir_lowering=False, debug=False, num_devices=N_CORES
    )
    enc = nc.dram_tensor("enc", [BL, DP, L], dt1, kind="ExternalInput").ap()
    hid = nc.dram_tensor("hid", [H, BL], F32R, kind="ExternalInput").ap()
    wet = nc.dram_tensor("wet", [DP, H], dt1, kind="ExternalInput").ap()
    wht = nc.dram_tensor("wht", [H, H], F32R, kind="ExternalInput").ap()
    bcol = nc.dram_tensor("bcol", [128, NHT], F32, kind="ExternalInput").ap()
    vcol = nc.dram_tensor("vcol", [128, NHT], F32 if dve2 else dt2, kind="ExternalInput").ap()
    ones = None
    vcolr = None
    if dve2:
        ones = nc.dram_tensor("ones", [128, 1], dt2, kind="ExternalInput").ap()
        vcolr = nc.dram_tensor("vcolr", [128, NHT], dt2, kind="ExternalInput").ap()
    out = nc.dram_tensor("out", [BL, L], F32, kind="ExternalOutput").ap()

    with tile.TileContext(nc) as tc:
        with (
            tc.tile_pool(name="const", bufs=1) as cpool,
            tc.tile_pool(name="encp", bufs=4) as epool,
            tc.tile_pool(name="energy", bufs=8) as gpool,
            tc.tile_pool(name="cb", bufs=2) as cbpool,
            tc.tile_pool(name="soft", bufs=1) as spool,
            tc.tile_pool(name="stage", bufs=4) as stpool,
            tc.tile_pool(name="ps1", bufs=6, space="PSUM") as ps1,
            tc.tile_pool(name="ps2", bufs=2, space="PSUM") as ps2,
            tc.tile_pool(name="ps3", bufs=2, space="PSUM") as ps3,
        ):
            # ---- PE warmup fuel: memset tile (no DMA dependency) so the
            # warmup starts at the Tensor engine's ~8us wake-up instead of
            # ~10us waiting out the wet0-DMA semaphore. bf16, not dt1:
            # DVE memset on a float32r tile fails codegen.
            wtile = cpool.tile([128, 512], BF16, tag="wtile", name="wtile")
            nc.vector.memset(wtile[:], 1.0)

            # ---- replicated constants: wet first (stage-1 weights), then
            # batch 0's enc tiles, then the small constants, then wht/hid —
            # so stage-1 for batch 0 can start ~5us earlier than when
            # enc[b0] queues behind every constant. ----
            wet_sb = []
            for kt in range(NKT):
                t = cpool.tile([128, H], dt1, tag=f"wet{kt}", name=f"wet{kt}")
                nc.sync.dma_start(t[:], wet[kt * 128 : (kt + 1) * 128, :])
                wet_sb.append(t)
            et0 = []
            for kt in range(NKT):
                t = epool.tile([128, L], dt1, tag=f"enc{kt}", name=f"enc{kt}_0")
                nc.sync.dma_start(t[:], enc[0, kt * 128 : (kt + 1) * 128, :])
                et0.append(t)
            bcol_sb = cpool.tile([128, NHT], F32, tag="bcol", name="bcol_sb")
            nc.sync.dma_start(bcol_sb[:], bcol[:, :])
            vcol_sb = cpool.tile([128, NHT], F32 if dve2 else dt2, tag="vcol", name="vcol_sb")
            nc.sync.dma_start(vcol_sb[:], vcol[:, :])
            ones128 = None
            vcolr_sb = None
            if dve2:
                ones128 = cpool.tile([128, 1], dt2, tag="ones128", name="ones128")
                nc.sync.dma_start(ones128[:], ones[:, :])
                vcolr_sb = cpool.tile([128, NHT], dt2, tag="vcolr", name="vcolr_sb")
                nc.sync.dma_start(vcolr_sb[:], vcolr[:, :])
            wht_sb = []
            for kt in range(4):
                t = cpool.tile([128, H], F32R, tag=f"wht{kt}", name=f"wht{kt}")
                nc.sync.dma_start(t[:], wht[kt * 128 : (kt + 1) * 128, :])
                wht_sb.append(t)
            hid_sb = []
            for kt in range(4):
                t = cpool.tile([128, BL], F32R, tag=f"hid{kt}", name=f"hid{kt}")
                nc.sync.dma_start(t[:], hid[kt * 128 : (kt + 1) * 128, :])
                hid_sb.append(t)

            for _rep in range(reps):
                # ---- PE warmup: high-duty junk matmuls from the engine
                # wake-up, so the HAM clock-gate reaches 8/8 before real
                # work (the N=16 c matmuls have ~3% array duty and never
                # warm it). 30 matmuls bridge until batch 0's enc tiles
                # land; batch 0's stage-1 group follows immediately.
                warm = ps1.tile([128, 512], F32, tag="ps1", name="warm")
                NW = 30
                for w in range(NW):
                    nc.tensor.matmul(
                        warm[:],
                        lhsT=wtile[:, 0:128],
                        rhs=wtile[:],
                        start=(w == 0),
                        stop=(w == NW - 1),
                    )

                # ---- batch 0, l-half 0: stage-1 matmuls BEFORE the c-block,
                # so the PE rolls from warmup straight into real work as soon
                # as enc[b0] lands (~14us); the tanh (needs cb) follows the
                # c-block inside the main loop.
                # kt-major so each enc[b0] k-tile's 4 matmuls issue as soon
                # as that tile's DMA lands (tiles arrive sequentially in the
                # prologue) instead of every h-group blocking on tile 4.
                pes00 = [
                    ps1.tile([128, 512], F32, tag="ps1", name=f"pe0_0_{ht}")
                    for ht in range(4)
                ]
                for kt in range(NKT):
                    for ht in range(4):
                        nc.tensor.matmul(
                            pes00[ht][:],
                            lhsT=wet_sb[kt][:, ht * 128 : (ht + 1) * 128],
                            rhs=et0[kt][:, 0:512],
                            start=(kt == 0),
                            stop=(kt == NKT - 1),
                        )

                # ---- c[h, b] = (hidden @ Wh.T).T + bias, per-partition h ----
                cb_sb = []
                for ht in range(4):
                    pc = ps1.tile([128, 512], F32, tag="ps1", name=f"pc{ht}")
                    for kt in range(4):
                        nc.tensor.matmul(
                            pc[:, :BL],
                            lhsT=wht_sb[kt][:, ht * 128 : (ht + 1) * 128],
                            rhs=hid_sb[kt][:],
                            start=(kt == 0),
                            stop=(kt == 3),
                        )
                    cbt = cbpool.tile([128, BL], F32, tag=f"cb{ht}", name=f"cb{ht}")
                    nc.vector.tensor_scalar_add(
                        cbt[:], pc[:, :BL], bcol_sb[:, ht : ht + 1]
                    )
                    cb_sb.append(cbt)

                scores_sb = spool.tile([BL, L], F32, tag="scores", name="scores_sb")

                # ---- main loop over local batches ----
                for b in range(BL):
                    if b == 0:
                        et = et0
                    else:
                        et = []
                        for kt in range(NKT):
                            t = epool.tile(
                                [128, L], dt1, tag=f"enc{kt}", name=f"enc{kt}_{b}"
                            )
                            nc.sync.dma_start(
                                t[:], enc[b, kt * 128 : (kt + 1) * 128, :]
                            )
                            et.append(t)

                    for lh in range(NLH):
                        ens = []
                        for ht in range(4):
                            if b == 0 and lh == 0:
                                pe_t = pes00[ht]
                            else:
                                pe_t = ps1.tile(
                                    [128, 512], F32, tag="ps1", name=f"pe{b}_{lh}_{ht}"
                                )
                                for kt in range(NKT):
                                    nc.tensor.matmul(
                                        pe_t[:],
                                        lhsT=wet_sb[kt][:, ht * 128 : (ht + 1) * 128],
                                        rhs=et[kt][:, lh * 512 : (lh + 1) * 512],
                                        start=(kt == 0),
                                        stop=(kt == NKT - 1),
                                    )
                            en_t = gpool.tile(
                                [128, 512], dt2, tag="en", name=f"en{b}_{lh}_{ht}"
                            )
                            nc.scalar.activation(
                                en_t[:], pe_t[:], AF.Tanh,
                                bias=cb_sb[ht][:, b : b + 1],
                            )
                            ens.append(en_t)
                        if dve2 and b < BL - 2:
                            # z[p, l] = sum_ht v_ht[p] * en_ht[p, l]  (DVE).
                            # Intermediates accumulate in plain f32; only the
                            # final tile is written as dt2 for the ones-matmul.
                            z = None
                            for ht in range(4):
                                zn = stpool.tile(
                                    [128, 512], dt2 if ht == 3 else F32, tag="z",
                                    name=f"z{b}_{lh}_{ht}", bufs=8,
                                )
                                if z is None:
                                    nc.vector.tensor_scalar_mul(
                                        zn[:], ens[ht][:], vcol_sb[:, ht : ht + 1]
                                    )
                                else:
                                    nc.vector.scalar_tensor_tensor(
                                        zn[:], ens[ht][:],
                                        vcol_sb[:, ht : ht + 1], z[:],
                                        AluOpType.mult, AluOpType.add,
                                    )
                                z = zn
                            # scores[l] = sum_p z[p, l]: one K=128 ones-matmul
                            ps_s = ps3.tile(
                                [1, 512], F32, tag="pss", name=f"ps_s{b}_{lh}"
                            )
                            nc.tensor.matmul(
                                ps_s[:], lhsT=ones128[:], rhs=z[:],
                                start=True, stop=True,
                            )
                        else:
                            vc = vcolr_sb if dve2 else vcol_sb
                            pspool = ps3 if dve2 else ps2
                            ps_s = pspool.tile(
                                [1, 512], F32, tag="pss", name=f"ps_s{b}_{lh}"
                            )
                            for ht in range(4):
                                nc.tensor.matmul(
                                    ps_s[:],
                                    lhsT=vc[:, ht : ht + 1],
                                    rhs=ens[ht][:],
                                    start=(ht == 0),
                                    stop=(ht == 3),
                                )
                        # stage psum scores out and park them batch-major
                        st = stpool.tile([1, 512], F32, tag="st", name=f"st{b}_{lh}")
                        nc.vector.tensor_copy(st[:], ps_s[:])
                        nc.sync.dma_start(
                            scores_sb[b : b + 1, lh * 512 : (lh + 1) * 512], st[:]
                        )

                # ---- one softmax over all local batches. No max-
                # subtraction: scores = v . tanh(...) are O(1) (|s| < 7),
                # so f32 exp cannot overflow and the result matches the
                # max-shifted softmax to ~1 ulp. Removes a serial DVE
                # reduce (~1.3us) from the tail chain.
                ex = spool.tile([BL, L], F32, tag="ex", name="ex")
                sm = spool.tile([BL, 1], F32, tag="sm", name="sm")
                nc.scalar.activation(
                    ex[:], scores_sb[:], AF.Exp,
                    accum_out=sm[:],
                )
                rc = spool.tile([BL, 1], F32, tag="rc", name="rc")
                nc.vector.reciprocal(rc[:], sm[:])
                oo = spool.tile([BL, L], F32, tag="oo", name="oo")
                nc.vector.tensor_scalar_mul(oo[:], ex[:], rc[:, 0:1])
                nc.sync.dma_start(out[:, :], oo[:])

    nc.compile()
    return nc


_cached_nc = None


def _prep_in_maps(hidden, encoder_outputs, W, b, v, np1=np.float32,
                  np2=np.float32):
    hidden = np.ascontiguousarray(hidden, dtype=np.float32)
    W = np.ascontiguousarray(W, dtype=np.float32)
    b = np.ascontiguousarray(b, dtype=np.float32)
    v = np.ascontiguousarray(v, dtype=np.float32)
    # (L, B, D) -> (B, D, L), zero-padded to DP on the d axis
    e = np.asarray(encoder_outputs, dtype=np.float32)
    encT = np.zeros((B, DP, L), dtype=np1)
    encT[:, :DE, :] = e.transpose(1, 2, 0).astype(np1)
    wet = np.zeros((DP, H), dtype=np1)
    wet[:DE] = W[:, H:].T.astype(np1)                   # We.T (padded)
    wht = np.ascontiguousarray(W[:, :H].T)              # (512, 512)
    bcol = np.ascontiguousarray(b.reshape(NHT, 128).T)  # (128, 4)
    vcol = np.ascontiguousarray(v.reshape(NHT, 128).T)  # (128, 4), f32
    ones = np.ones((128, 1), dtype=np2)
    in_maps = []
    for c in range(N_CORES):
        sl = slice(c * BL, (c + 1) * BL)
        in_maps.append(
            {
                "enc": encT[sl],
                "hid": np.ascontiguousarray(hidden[sl].T),
                "wet": wet,
                "wht": wht,
                "bcol": bcol,
                "vcol": vcol,
                "ones": ones,
                "vcolr": vcol.astype(np2),
            }
        )
    return in_maps


def kernel(hidden, encoder_outputs, W, b, v):
    global _cached_nc
    if _cached_nc is None:
        _cached_nc = build(reps=1)
    in_maps = _prep_in_maps(hidden, encoder_outputs, W, b, v)
    res = bass_utils.run_bass_kernel_spmd(
        _cached_nc, in_maps, core_ids=list(range(N_CORES))
    )
    outs = np.concatenate([res.results[c]["out"] for c in range(N_CORES)], axis=0)
    return outs[:, None, :].astype(np.float32)

